# revision 1
# baseline (speedup 1.0000x reference)
"""Trainium2 Bass kernel for nn_Attention (dense transformer attention).

Math (per batch n, head h):
  q' = q_h @ Wq.T ; k' = k_h @ Wk.T ; v' = v_h @ Wv.T
  S = (q' k'^T)/32 ; P = softmax_k(S) ; out_h = P v'
  final = concat_h(out_h) @ Wout.T + bout

Device-side reformulation (all via associativity, exact in real arithmetic):
  S   = Q @ Wc @ K^T      with Wc = (Wq.T @ Wk)/32   (K unprojected!)
  U^T = [V | 1]^T @ exp(S)^T   -> rows 0..63 = V^T exp(S)^T, row 64 = softmax denoms
  out_h^T = (Wv @ U^T[0:64]) / denom    (Wv projection moved after attention)
  final^T = Wout @ attn^T + bout

Sharding: sequence-parallel over the 2048 queries -> 8 cores x 256 queries.
Each core reads full (transposed) keys + full values, its query slice, and
writes its 256-query slice of the final output (transposed). Host just
concatenates + transposes back - no collectives, no host-side reduction.

Everything the device consumes is laid out on the host so that every DMA is
a natural contiguous/strided read and no on-device transposes are needed:
  kT  (2, 1024, 2048)  keys^T   (embed-major)
  qT  (2, 1024, 256)   query^T slice
  v   (2, 2048, 1024)  values   (token-major, raw)
  wqk2 (128, 128)      blockdiag(Wc, Wc) so Q'' for a head pair is one
                       full-width matmul (TRN2 rejects fp32r matmuls with
                       tile_position col offsets)
  wvT  (64, 64)        Wv.T
  woutT (1024, 1024)   Wout.T   (e-major)
  bias2 (128, 8)       bout.reshape(8,128).T
"""

import sys

for p in ("/opt/trn_rl_repo",):
    if p not in sys.path:
        sys.path.insert(0, p)

import numpy as np

N = 2
L = 2048
E = 1024
H = 16
D = 64
NCORES = 8
LQ = L // NCORES          # 256 queries per core
NPAIR = H // 2            # 8 head-pairs per batch
NCHUNK = L // 128         # 16 key chunks of 128 tokens
import os as _os
REPEAT = int(_os.environ.get("BASS_KERNEL_REPEAT", "1"))

_F32R = None


def build_nc():
    import concourse.bass as bass
    import concourse.bacc as bacc
    import concourse.mybir as mybir
    import concourse.tile as tile

    f32 = mybir.dt.float32
    f32r = mybir.dt.float32r
    EXP = mybir.ActivationFunctionType.Exp
    MUL = mybir.AluOpType.mult
    ADD = mybir.AluOpType.add

    nc = bacc.Bacc(None, target_bir_lowering=False)

    kT = nc.dram_tensor("kT", [N, E, L], f32r, kind="ExternalInput")
    v = nc.dram_tensor("v", [N, L, E], f32r, kind="ExternalInput")
    qT = nc.dram_tensor("qT", [N, E, LQ], f32r, kind="ExternalInput")
    wqk2 = nc.dram_tensor("wqk2", [128, 128], f32r, kind="ExternalInput")
    wvT = nc.dram_tensor("wvT", [D, D], f32r, kind="ExternalInput")
    woutT = nc.dram_tensor("woutT", [E, E], f32r, kind="ExternalInput")
    bias2 = nc.dram_tensor("bias2", [128, E // 128], f32, kind="ExternalInput")
    ones_d = nc.dram_tensor("ones_d", [128, 128], f32r, kind="ExternalInput")
    outT = nc.dram_tensor("outT", [N, E, LQ], f32, kind="ExternalOutput")

    with tile.TileContext(nc) as tc:
        with (
            tc.tile_pool(name="const", bufs=1) as const,
            tc.tile_pool(name="io", bufs=2) as io,
            tc.tile_pool(name="work", bufs=3) as work,
            tc.tile_pool(name="psT", bufs=2, space="PSUM") as psT,
            tc.tile_pool(name="puT", bufs=2, space="PSUM") as puT,
            tc.tile_pool(name="psmall", bufs=2, space="PSUM") as psmall,
        ):
            # --- persistent constants ---
            wqk2_sb = const.tile([128, 128], f32r)
            nc.sync.dma_start(wqk2_sb, wqk2[:, :])
            wvT_sb = const.tile([D, D], f32r)
            nc.sync.dma_start(wvT_sb, wvT[:, :])
            wout_sb = const.tile([128, E // 128, E], f32r)
            nc.sync.dma_start(wout_sb, woutT.rearrange("(ec p) o -> p ec o", p=128))
            bias_sb = const.tile([128, E // 128], f32)
            nc.sync.dma_start(bias_sb, bias2[:, :])
            ones_sb = const.tile([128, 128], f32r)
            nc.sync.dma_start(ones_sb, ones_d[:, :])

            import contextlib

            rep_ctx = (
                tc.For_i(0, REPEAT, 1) if REPEAT > 1 else contextlib.nullcontext()
            )
            with rep_ctx:
              for n in range(N):
                attn_sb = io.tile([128, NPAIR, LQ], f32r, tag="attn")
                for h2 in range(NPAIR):
                    # --- loads for this head pair (heads 2*h2, 2*h2+1) ---
                    kT2 = io.tile([128, L], f32r, tag="kT2")
                    nc.sync.dma_start(kT2, kT[n, 128 * h2 : 128 * (h2 + 1), :])
                    qT2 = io.tile([128, LQ], f32r, tag="qT2")
                    nc.sync.dma_start(qT2, qT[n, 128 * h2 : 128 * (h2 + 1), :])
                    v2 = io.tile([128, NCHUNK, 130], f32r, tag="v2")
                    vsrc = v[n].rearrange("(c p) e -> p c e", p=128)
                    nc.sync.dma_start(
                        v2[:, :, 0:64], vsrc[:, :, 128 * h2 : 128 * h2 + 64]
                    )
                    nc.sync.dma_start(
                        v2[:, :, 65:129], vsrc[:, :, 128 * h2 + 64 : 128 * h2 + 128]
                    )
                    nc.sync.dma_start(v2[:, :, 64:65], ones_d[:, 0:NCHUNK])
                    nc.sync.dma_start(v2[:, :, 129:130], ones_d[:, 0:NCHUNK])

                    # --- Q'' = (Q @ Wc)^T for both heads -> [128, LQ] ---
                    pq = psmall.tile([128, LQ], f32, tag="small")
                    nc.tensor.matmul(pq, wqk2_sb, qT2, start=True, stop=True)
                    q2sb = work.tile([128, LQ], f32r, tag="q2sb")
                    nc.vector.tensor_copy(q2sb, pq)

                    r2_sb = work.tile([65, 2, LQ], f32r, tag="r2")
                    # S^T for both heads interleaved: the PE runs the two
                    # heads' matmuls concurrently in disjoint 64-row groups.
                    # 4 chunks per PSUM tile, double-buffered for ACT overlap.
                    expS0 = work.tile([128, NCHUNK, LQ], f32r, tag="expS")
                    expS1 = work.tile([128, NCHUNK, LQ], f32r, tag="expS")
                    exps = (expS0, expS1)
                    for rr in range(4):
                        sTs = []
                        for hh in range(2):
                            hs = slice(64 * hh, 64 * hh + 64)
                            sT = psT.tile([128, 4, LQ], f32, tag="sT")
                            sTs.append(sT)
                            for c in range(4):
                                ch = rr * 4 + c
                                nc.tensor.matmul(
                                    sT[:, c, :],
                                    kT2[hs, 128 * ch : 128 * (ch + 1)],
                                    q2sb[hs, :],
                                    start=True, stop=True,
                                )
                        for hh in range(2):
                            nc.scalar.activation(
                                exps[hh][:, rr * 4 : rr * 4 + 4, :],
                                sTs[hh][:, :, :], EXP,
                            )
                    # --- U^T = [V|1]^T @ expS^T (accumulate over chunks) ---
                    ups = []
                    for hh in range(2):
                        uT = puT.tile([65, LQ], f32, tag="uT")
                        for ch in range(NCHUNK):
                            nc.tensor.matmul(
                                uT,
                                v2[:, ch, 65 * hh : 65 * hh + 65],
                                exps[hh][:, ch, :],
                                start=(ch == 0), stop=(ch == NCHUNK - 1),
                            )
                        u_sb = work.tile([65, LQ], f32r, tag="u_sb")
                        nc.vector.tensor_copy(u_sb, uT)
                        # reciprocal of denominators (row 64)
                        with nc.allow_low_precision("feeds fp32r bcast matmul"):
                            nc.vector.reciprocal(r2_sb[64:65, hh, :], u_sb[64:65, :])
                        # --- project with Wv: U'^T = Wv @ U^T ---
                        up = puT.tile([65, LQ], f32, tag="uT")
                        ups.append(up)
                        nc.tensor.matmul(
                            up[0:64, :], wvT_sb, u_sb[0:64, :],
                            start=True, stop=True,
                        )

                    # --- broadcast 1/denom across 64 partitions via PE outer ---
                    pb = psmall.tile([64, 2 * LQ], f32, tag="small")
                    nc.tensor.matmul(
                        pb, ones_sb[64:65, 0:64], r2_sb[64:65, :, :],
                        start=True, stop=True,
                    )
                    b_sb = work.tile([64, 2, LQ], f32, tag="b_sb")
                    nc.vector.tensor_copy(b_sb, pb)
                    # --- normalize and place into attn^T tile ---
                    # head 0 -> partitions 0-63 directly
                    nc.vector.tensor_tensor(
                        attn_sb[0:64, h2, :], ups[0][0:64, :], b_sb[:, 0, :], MUL
                    )
                    # head 1 -> via bounce + SBUF->SBUF DMA (partition shift)
                    bounce = work.tile([64, LQ], f32r, tag="bounce")
                    nc.vector.tensor_tensor(
                        bounce, ups[1][0:64, :], b_sb[:, 1, :], MUL
                    )
                    nc.sync.dma_start(attn_sb[64:128, h2, :], bounce)

                # --- fc_out: final^T = Wout @ attn^T + bout ---
                for oc in range(E // 128):
                    po = psmall.tile([128, LQ], f32, tag="small")
                    for ec in range(E // 128):
                        nc.tensor.matmul(
                            po,
                            wout_sb[:, ec, 128 * oc : 128 * (oc + 1)],
                            attn_sb[:, ec, :],
                            start=(ec == 0), stop=(ec == E // 128 - 1),
                        )
                    o_sb = work.tile([128, LQ], f32, tag="o_sb")
                    nc.vector.tensor_tensor(
                        o_sb, po,
                        bias_sb[:, oc : oc + 1].to_broadcast((128, LQ)),
                        ADD,
                    )
                    nc.sync.dma_start(outT[n, 128 * oc : 128 * (oc + 1), :], o_sb)

    nc.compile()
    return nc


def shard_inputs(values, keys, query, Wv, Wk, Wq, Wout, bout):
    f = np.float32
    values = np.ascontiguousarray(np.asarray(values), dtype=f)
    kT_full = np.ascontiguousarray(np.asarray(keys).transpose(0, 2, 1), dtype=f)
    qT_full = np.ascontiguousarray(np.asarray(query).transpose(0, 2, 1), dtype=f)
    Wv, Wk, Wq, Wout, bout = (np.asarray(x, dtype=f) for x in (Wv, Wk, Wq, Wout, bout))
    Wc = (Wq.T @ Wk) / np.float32(np.sqrt(E))
    wqk2 = np.zeros((128, 128), dtype=f)
    wqk2[0:64, 0:64] = Wc
    wqk2[64:128, 64:128] = Wc
    wvT = np.ascontiguousarray(Wv.T, dtype=f)
    woutT = np.ascontiguousarray(Wout.T, dtype=f)
    bias2 = np.ascontiguousarray(bout.reshape(E // 128, 128).T, dtype=f)
    ones = np.ones((128, 128), dtype=f)
    in_maps = []
    for c in range(NCORES):
        in_maps.append({
            "kT": kT_full,
            "v": values,
            "qT": np.ascontiguousarray(qT_full[:, :, c * LQ : (c + 1) * LQ]),
            "wqk2": wqk2,
            "wvT": wvT,
            "woutT": woutT,
            "bias2": bias2,
            "ones_d": ones,
        })
    return in_maps


def unshard(results):
    slabs = [np.asarray(r["outT"]).transpose(0, 2, 1) for r in results]
    return np.ascontiguousarray(np.concatenate(slabs, axis=1)).astype(np.float32)


def run_spmd(in_maps, **kwargs):
    from concourse.bass_utils import run_bass_kernel_spmd

    nc = build_nc()
    res = run_bass_kernel_spmd(nc, in_maps, core_ids=list(range(NCORES)), **kwargs)
    return nc, res


def kernel(**inputs):
    in_maps = shard_inputs(
        inputs["values"], inputs["keys"], inputs["query"],
        inputs["Wv"], inputs["Wk"], inputs["Wq"],
        inputs["Wout"], inputs["bout"],
    )
    _, res = run_spmd(in_maps)
    return unshard(res.results)


if __name__ == "__main__":
    rng = np.random.default_rng(0)
    ins = {
        "values": rng.standard_normal((N, L, E), dtype=np.float32),
        "keys": rng.standard_normal((N, L, E), dtype=np.float32),
        "query": rng.standard_normal((N, L, E), dtype=np.float32),
        "Wv": rng.standard_normal((D, D), dtype=np.float32) / 8,
        "Wk": rng.standard_normal((D, D), dtype=np.float32) / 8,
        "Wq": rng.standard_normal((D, D), dtype=np.float32) / 8,
        "Wout": rng.standard_normal((E, E), dtype=np.float32) / 32,
        "bout": rng.standard_normal((E,), dtype=np.float32) * 0.01,
    }
    out = kernel(**ins)
    print("out", out.shape, out.dtype, float(np.abs(out).max()))



# revision 6
# speedup vs baseline: 40.3348x; 40.3348x over previous
"""Trainium2 Bass kernel for nn_Attention (dense transformer attention).

Math (per batch n, head h):
  q' = q_h @ Wq.T ; k' = k_h @ Wk.T ; v' = v_h @ Wv.T
  S = (q' k'^T)/32 ; P = softmax_k(S) ; out_h = P v'
  final = concat_h(out_h) @ Wout.T + bout

Device-side reformulation (associativity, exact in real arithmetic):
  S   = Q @ Wc @ K^T      with Wc = (Wq.T @ Wk)/32   (K unprojected!)
  U^T = [V | 1]^T @ exp(S)^T   -> rows 0..63 = V^T exp(S)^T, row 64 = denoms
  out_h^T = (Wv @ U^T[0:64]) / denom    (Wv projection after attention)
  final^T = Wout @ attn^T + bout

Numerics: everything that feeds the PE is bf16 (|S| <= ~2.5 so exp is tame;
measured end-to-end absmax rel err ~4e-3, tolerance 2e-2). PSUM accumulation
stays f32, final output written f32.

Sharding: sequence-parallel over the 2048 queries -> 8 cores x 256 queries.

Schedule: the 16 (batch, head-pair) slots are software-pipelined. In slot p
the PE computes S^T(p) (feeding ACT's exp, the bottleneck engine) and then
the U^T/Wv/normalize flush of slot p-1, whose exp outputs finished during
slot p-1. fc_out for batch n runs in slot (n+1, 0) after that slot's S^T,
so ACT keeps a full pair of exp work during fc_out's matmuls.

Host-side packing makes every DMA big-descriptor and cuts the input-tensor
count to 4:
  kT    (N, E, L)                bf16 keys^T        4 KiB descriptors
  qT    (N, E, LQ)               bf16 query^T slice 512 B descriptors
  vpack (N, 128, NCHUNK, H, 65)  bf16 [V | 1] token-partition-major; one
                                 8 KiB contiguous descriptor per partition
                                 per 4-chunk load
  wpack (128, 8584)              bf16 all weights in one tensor:
        [:, 0:128]    blockdiag(Wc, Wc)
        [0:64, 128:384]  [Wv.T | 0] and [0 | Wv.T]  (head0/head1 PE placement)
        [:, 384:392]  bias (bout partition-major)
        [:, 392:8584] Wout^T as [128, ec, o]
"""

import sys

for p in ("/opt/trn_rl_repo",):
    if p not in sys.path:
        sys.path.insert(0, p)

import numpy as np

N = 2
L = 2048
E = 1024
H = 16
D = 64
NCORES = 8
LQ = L // NCORES          # 256 queries per core
NPAIR = H // 2            # 8 head-pairs per batch
NCHUNK = L // 128         # 16 key chunks of 128 tokens
WCOL_WV = 128
WCOL_BIAS = 384
WCOL_WOUT = 392
WCOLS = 392 + E * (E // 128)

import os as _os
REPEAT = int(_os.environ.get("BASS_KERNEL_REPEAT", "1"))


def build_nc():
    import concourse.bass as bass
    import concourse.bacc as bacc
    import concourse.mybir as mybir
    import concourse.tile as tile

    f32 = mybir.dt.float32
    bf16 = mybir.dt.bfloat16
    EXP = mybir.ActivationFunctionType.Exp
    MUL = mybir.AluOpType.mult
    ADD = mybir.AluOpType.add

    nc = bacc.Bacc(None, target_bir_lowering=False)

    kT = nc.dram_tensor("kT", [N, E, L], bf16, kind="ExternalInput")
    qT = nc.dram_tensor("qT", [N, E, LQ], bf16, kind="ExternalInput")
    vpack = nc.dram_tensor("vpack", [N, 128, NCHUNK, H, D + 1], bf16,
                           kind="ExternalInput")
    wpack = nc.dram_tensor("wpack", [128, WCOLS], bf16, kind="ExternalInput")
    outT = nc.dram_tensor("outT", [N, E, LQ], f32, kind="ExternalOutput")

    with tile.TileContext(nc) as tc:
        with (
            tc.tile_pool(name="const", bufs=1) as const,
            tc.tile_pool(name="vio", bufs=2) as vio,
            tc.tile_pool(name="io", bufs=2) as io,
            tc.tile_pool(name="work", bufs=3) as work,
            tc.tile_pool(name="expp", bufs=4) as expp,
            tc.tile_pool(name="psT", bufs=2, space="PSUM") as psT,
            tc.tile_pool(name="pu", bufs=2, space="PSUM") as pu,
            tc.tile_pool(name="psmall", bufs=2, space="PSUM") as psmall,
        ):
            # --- persistent constants: one DMA for all weights ---
            wpack_sb = const.tile([128, WCOLS], bf16)
            nc.sync.dma_start(wpack_sb, wpack[:, :])
            wqk2_sb = wpack_sb[:, 0:128]

            bias_sb = const.tile([128, E // 128], f32)
            nc.vector.tensor_copy(bias_sb, wpack_sb[:, WCOL_BIAS:WCOL_WOUT])
            ones_sb = const.tile([128, 128], bf16)
            nc.vector.memset(ones_sb, 1.0)

            import contextlib

            def load_pair(n, h2):
                kT2 = io.tile([128, L], bf16, tag="kT2")
                nc.sync.dma_start(kT2, kT[n, 128 * h2 : 128 * (h2 + 1), :])
                qT2 = io.tile([128, LQ], bf16, tag="qT2")
                nc.sync.dma_start(qT2, qT[n, 128 * h2 : 128 * (h2 + 1), :])
                return kT2, qT2

            def load_v(n):
                # 4 DMAs of 4 chunks each so early U^T chunks never wait on
                # the full 8 MiB load
                v_sb = vio.tile([128, NCHUNK, H, D + 1], bf16, tag="v",
                                name=f"v_sb_{n}")
                for g in range(4):
                    nc.sync.dma_start(
                        v_sb[:, 4 * g : 4 * (g + 1)], vpack[n, :, 4 * g : 4 * (g + 1)]
                    )
                return v_sb

            def score_phase(kT2, qT2):
                """Q'' projection, S^T matmuls and exp for one head pair."""
                pq = psmall.tile([128, LQ], f32, tag="small")
                nc.tensor.matmul(pq, wqk2_sb, qT2, start=True, stop=True)
                q2sb = work.tile([128, LQ], bf16, tag="q2sb")
                with nc.allow_low_precision("bf16 attention pipeline"):
                    nc.vector.tensor_copy(q2sb, pq)

                expS0 = expp.tile([128, NCHUNK, LQ], bf16, tag="expS")
                expS1 = expp.tile([128, NCHUNK, LQ], bf16, tag="expS")
                exps = (expS0, expS1)
                for rr in range(4):
                    sTs = []
                    for hh in range(2):
                        hs = slice(64 * hh, 64 * hh + 64)
                        sT = psT.tile([128, 4, LQ], f32, tag="sT")
                        sTs.append(sT)
                        for c in range(4):
                            ch = rr * 4 + c
                            nc.tensor.matmul(
                                sT[:, c, :],
                                kT2[hs, 128 * ch : 128 * (ch + 1)],
                                q2sb[hs, :],
                                start=True, stop=True,
                            )
                    for hh in range(2):
                        with nc.allow_low_precision("bf16 exp(S)"):
                            nc.scalar.activation(
                                exps[hh][:, rr * 4 : rr * 4 + 4, :],
                                sTs[hh][:, :, :], EXP,
                            )
                return exps

            def flush_pair(n, h2, v_sb, exps, attn_sb):
                """U^T accumulate, Wv projection, softmax normalize into
                attn_sb for a pair whose exp outputs are ready."""
                r2_sb = work.tile([65, 2, LQ], bf16, tag="r2")
                u_sbs = []
                for hh in range(2):
                    uT = pu.tile([65, LQ], f32, tag="uT")
                    for ch in range(NCHUNK):
                        nc.tensor.matmul(
                            uT,
                            v_sb[:, ch, 2 * h2 + hh, :],
                            exps[hh][:, ch, :],
                            start=(ch == 0), stop=(ch == NCHUNK - 1),
                        )
                    u_sb = work.tile([65, LQ], bf16, tag="u_sb")
                    u_sbs.append(u_sb)
                    with nc.allow_low_precision("bf16 attention pipeline"):
                        nc.vector.tensor_copy(u_sb, uT)
                        nc.vector.reciprocal(r2_sb[64:65, hh, :], u_sb[64:65, :])
                # Wv projection, head hh placed at partitions 64*hh..64*hh+63
                # via the zero-padded [Wv.T|0]/[0|Wv.T] stationary operands
                up = pu.tile([128, LQ], f32, tag="uT")
                for hh in range(2):
                    nc.tensor.matmul(
                        up,
                        wpack_sb[0:64, WCOL_WV + 128 * hh : WCOL_WV + 128 * (hh + 1)],
                        u_sbs[hh][0:64, :],
                        start=(hh == 0), stop=(hh == 1),
                    )
                # broadcast 1/denom across partitions via PE outer product
                pb = psmall.tile([128, 2, LQ], f32, tag="small")
                nc.tensor.matmul(
                    pb, ones_sb[64:65, :], r2_sb[64:65, :, :],
                    start=True, stop=True,
                )
                b_sb = work.tile([128, 2, LQ], bf16, tag="b_sb")
                with nc.allow_low_precision("bf16 attention pipeline"):
                    nc.vector.tensor_copy(b_sb, pb)
                    nc.vector.tensor_tensor(
                        attn_sb[0:64, h2, :], up[0:64, :], b_sb[0:64, 0, :], MUL,
                    )
                    nc.vector.tensor_tensor(
                        attn_sb[64:128, h2, :], up[64:128, :], b_sb[64:128, 1, :],
                        MUL,
                    )

            def fc_out(n, attn_sb):
                for oc in range(E // 128):
                    po = psmall.tile([128, LQ], f32, tag="small")
                    for ec in range(E // 128):
                        nc.tensor.matmul(
                            po,
                            wpack_sb[:, WCOL_WOUT + E * ec + 128 * oc
                                     : WCOL_WOUT + E * ec + 128 * (oc + 1)],
                            attn_sb[:, ec, :],
                            start=(ec == 0), stop=(ec == E // 128 - 1),
                        )
                    o_sb = work.tile([128, LQ], f32, tag="o_sb")
                    nc.vector.tensor_tensor(
                        o_sb, po,
                        bias_sb[:, oc : oc + 1].to_broadcast((128, LQ)),
                        ADD,
                    )
                    nc.sync.dma_start(outT[n, 128 * oc : 128 * (oc + 1), :], o_sb)

            rep_ctx = (
                tc.For_i(0, REPEAT, 1) if REPEAT > 1 else contextlib.nullcontext()
            )
            with rep_ctx:
                slots = [(n, h2) for n in range(N) for h2 in range(NPAIR)]
                v_sbs = {}
                attn_sbs = {}
                v_sbs[0] = load_v(0)
                loaded = load_pair(*slots[0])
                prev = None
                for idx, (n, h2) in enumerate(slots):
                    if h2 == 0:
                        attn_sbs[n] = io.tile([128, NPAIR, LQ], bf16, tag="attn",
                                              name=f"attn_sb_{n}")
                    kT2, qT2 = loaded
                    if idx + 1 < len(slots):
                        loaded = load_pair(*slots[idx + 1])
                    if n == 0 and h2 == 4:
                        v_sbs[1] = load_v(1)
                    exps = score_phase(kT2, qT2)
                    if prev is not None:
                        pn, ph2, pexps = prev
                        flush_pair(pn, ph2, v_sbs[pn], pexps, attn_sbs[pn])
                        if ph2 == NPAIR - 1:
                            fc_out(pn, attn_sbs[pn])
                    prev = (n, h2, exps)
                pn, ph2, pexps = prev
                flush_pair(pn, ph2, v_sbs[pn], pexps, attn_sbs[pn])
                fc_out(pn, attn_sbs[pn])

    nc.compile()
    return nc


def shard_inputs(values, keys, query, Wv, Wk, Wq, Wout, bout):
    import ml_dtypes

    bf16 = ml_dtypes.bfloat16
    f = np.float32
    values = np.asarray(values, dtype=f)
    keys = np.asarray(keys, dtype=f)
    query = np.asarray(query, dtype=f)
    Wv, Wk, Wq, Wout, bout = (np.asarray(x, dtype=f) for x in (Wv, Wk, Wq, Wout, bout))

    kT_full = np.ascontiguousarray(keys.transpose(0, 2, 1)).astype(bf16)
    qT_full = np.ascontiguousarray(query.transpose(0, 2, 1)).astype(bf16)

    # [V | 1] token-partition-major: vpack[n, p, c, h, :] =
    #   [values[n, c*128+p, h*64:(h+1)*64], 1]
    vpack = np.ones((N, 128, NCHUNK, H, D + 1), dtype=bf16)
    vr = values.reshape(N, NCHUNK, 128, H, D).transpose(0, 2, 1, 3, 4)
    vpack[:, :, :, :, 0:D] = vr.astype(bf16)

    Wc = (Wq.T @ Wk) / np.float32(np.sqrt(E))
    wpack = np.zeros((128, WCOLS), dtype=bf16)
    wpack[0:64, 0:64] = Wc.astype(bf16)
    wpack[64:128, 64:128] = Wc.astype(bf16)
    wvT = Wv.T.astype(bf16)
    wpack[0:64, WCOL_WV : WCOL_WV + 64] = wvT
    wpack[0:64, WCOL_WV + 192 : WCOL_WV + 256] = wvT
    wpack[:, WCOL_BIAS:WCOL_WOUT] = (
        bout.reshape(E // 128, 128).T.astype(bf16)
    )
    # wout block: [p, ec*E + o] = Wout.T[ec*128 + p, o]
    woutT = np.ascontiguousarray(Wout.T).astype(bf16)
    wpack[:, WCOL_WOUT:] = (
        woutT.reshape(E // 128, 128, E).transpose(1, 0, 2).reshape(128, -1)
    )

    in_maps = []
    for c in range(NCORES):
        in_maps.append({
            "kT": kT_full,
            "qT": np.ascontiguousarray(qT_full[:, :, c * LQ : (c + 1) * LQ]),
            "vpack": vpack,
            "wpack": wpack,
        })
    return in_maps


def unshard(results):
    slabs = [np.asarray(r["outT"]).transpose(0, 2, 1) for r in results]
    return np.ascontiguousarray(np.concatenate(slabs, axis=1)).astype(np.float32)


def run_spmd(in_maps, **kwargs):
    from concourse.bass_utils import run_bass_kernel_spmd

    nc = build_nc()
    res = run_bass_kernel_spmd(nc, in_maps, core_ids=list(range(NCORES)), **kwargs)
    return nc, res


def kernel(**inputs):
    in_maps = shard_inputs(
        inputs["values"], inputs["keys"], inputs["query"],
        inputs["Wv"], inputs["Wk"], inputs["Wq"],
        inputs["Wout"], inputs["bout"],
    )
    _, res = run_spmd(in_maps)
    return unshard(res.results)


if __name__ == "__main__":
    rng = np.random.default_rng(0)
    ins = {
        "values": rng.standard_normal((N, L, E), dtype=np.float32),
        "keys": rng.standard_normal((N, L, E), dtype=np.float32),
        "query": rng.standard_normal((N, L, E), dtype=np.float32),
        "Wv": rng.standard_normal((D, D), dtype=np.float32) / 8,
        "Wk": rng.standard_normal((D, D), dtype=np.float32) / 8,
        "Wq": rng.standard_normal((D, D), dtype=np.float32) / 8,
        "Wout": rng.standard_normal((E, E), dtype=np.float32) / 32,
        "bout": rng.standard_normal((E,), dtype=np.float32) * 0.01,
    }
    out = kernel(**ins)
    print("out", out.shape, out.dtype, float(np.abs(out).max()))
